# revision 14
# baseline (speedup 1.0000x reference)
"""Bass/Trainium2 kernel for nn_GreedyMatcher: batched PDHG LP solver.

Reference computation (per batch sample b):
    B = X.reshape(bs, 128); Wb = broadcast(W)
    x0 = y0 = 0, xbar0 = 0
    repeat 100x:
        y   = relu(y + sigma*(xbar @ S.T - B))
        x'  = relu(x + tau*(W - y @ S))
        xbar = 2x' - x ; x = x'
    return x  [bs, 2048]

Strategy: pure data parallel over batch (256 -> 32 per core, 8 cores).
Per-core state is kept struct-major in SBUF: Xsb[p, 32*m + b] = x[b, 128*m + p]
so both matmuls per iteration run with K=128 chunks on the tensor engine.
The extrapolation xbar is never materialized: with V_t = S @ x_t^T,
S @ xbar_t^T = 2 V_t - V_{t-1}, and the dual update folds into a single
carried tensor g_t = y_t - sigma*V_{t-1} - sigma*B^T:
    v      = (2 sigma V_t) + g_t          # psum + g
    y_.    = relu(v)
    g_{t+1}= y_. - sigma*V_t - sigma*B^T
    x_{t+1}= relu(x_t + tau - tau*(S^T y_.))   (W == ones fast path)
"""

import sys
import os

sys.path.insert(0, "/opt/trn_rl_repo")

import numpy as np

N_CORES = 8
BATCH = 256
BS = BATCH // N_CORES  # 32 per core
N_HOS = 8
N_TYPES = 16
M_CONS = N_HOS * N_TYPES  # 128 constraints
N_STRUCTS = 2048
N_CHUNKS = N_STRUCTS // 128  # 16
N_ITERS = 100
N_STREAMS = 1

_CACHE = {}


def _spec_norm_f32(S: np.ndarray) -> np.float32:
    """Mimic reference._spec_norm in float32 numpy."""
    S = S.astype(np.float32)
    v = np.ones((S.shape[1],), np.float32)
    v = v / np.float32(np.linalg.norm(v))
    for _ in range(30):
        u = S @ v
        u = u / (np.float32(np.linalg.norm(u)) + np.float32(1e-12))
        v = S.T @ u
        v = v / (np.float32(np.linalg.norm(v)) + np.float32(1e-12))
    return np.float32(np.linalg.norm(S @ v))


def _get_compiled(tau: float, sigma: float, mm_dtype_name: str, w_is_ones: bool):
    key = (round(float(tau), 12), round(float(sigma), 12), mm_dtype_name, w_is_ones)
    if key in _CACHE:
        return _CACHE[key]
    nc = _build_real(mm_dtype_name, w_is_ones, float(tau), float(sigma))
    nc.compile()
    _CACHE[key] = nc
    return nc


def _build_real(mm_dtype_name: str, w_is_ones: bool, tau: float, sigma: float):
    import concourse.bacc as bacc
    import concourse.tile as tile
    import concourse.mybir as mybir
    from contextlib import ExitStack

    f32 = mybir.dt.float32
    mmdt = getattr(mybir.dt, mm_dtype_name)
    ALU = mybir.AluOpType
    ACT = mybir.ActivationFunctionType

    nc = bacc.Bacc(None, target_bir_lowering=False)

    STs_d = nc.dram_tensor("STs", [128, N_STRUCTS], mmdt, kind="ExternalInput")
    Ss_d = nc.dram_tensor("Ss", [128, N_STRUCTS], mmdt, kind="ExternalInput")
    EYE_d = nc.dram_tensor("EYE", [128, 128], mmdt, kind="ExternalInput")
    BH_d = nc.dram_tensor("BH", [128, BS], f32, kind="ExternalInput")
    GHI0_d = nc.dram_tensor("GHI0", [128, BS], mmdt, kind="ExternalInput")
    GLO0_d = nc.dram_tensor("GLO0", [128, BS], mmdt, kind="ExternalInput")
    if not w_is_ones:
        TW_d = nc.dram_tensor("TW", [128, N_CHUNKS * BS], f32, kind="ExternalInput")
    XO_d = nc.dram_tensor("XOUT", [128, N_CHUNKS * BS], f32, kind="ExternalOutput")

    FD = N_CHUNKS * BS  # 512
    HF = FD // 2

    two_sigma = float(2.0 * sigma)

    with tile.TileContext(nc) as tc:
        with ExitStack() as ctx:
            const = ctx.enter_context(tc.tile_pool(name="const", bufs=1))
            state = ctx.enter_context(tc.tile_pool(name="state", bufs=1))
            tmp = ctx.enter_context(tc.tile_pool(name="tmp", bufs=3))
            psum = ctx.enter_context(tc.tile_pool(name="psum", bufs=2, space="PSUM"))

            STs = const.tile([128, N_STRUCTS], mmdt, tag="STs")
            Ss = const.tile([128, N_STRUCTS], mmdt, tag="Ss")
            EYE = const.tile([128, 128], mmdt, tag="EYE")
            Bt = const.tile([128, BS], f32, tag="Bt")
            nc.sync.dma_start(STs[:], STs_d[:])
            nc.sync.dma_start(Ss[:], Ss_d[:])
            nc.sync.dma_start(EYE[:], EYE_d[:])
            nc.sync.dma_start(Bt[:], BH_d[:])
            if not w_is_ones:
                TW = const.tile([128, FD], f32, tag="TW")
                nc.sync.dma_start(TW[:], TW_d[:])

            taub = const.tile([128, 1], f32, tag="taub")
            zb = const.tile([128, 1], f32, tag="zb")
            nc.gpsimd.memset(taub[:], float(tau))
            nc.gpsimd.memset(zb[:], 0.0)

            # state: xt = x + tau*W (fp32), x16 = fp16(x), gm = g/sigma (fp32)
            # plus fp16 hi/lo pair of gm for the PSUM identity fold.
            xt = [state.tile([128, FD], f32, name=f"xt_{i}", tag=f"xt_{i}")
                  for i in range(2)]
            x16 = [state.tile([128, FD], mmdt, name=f"x16_{i}", tag=f"x16_{i}")
                   for i in range(2)]
            gm = [state.tile([128, BS], f32, name=f"gm_{i}", tag=f"gm_{i}")
                  for i in range(2)]
            Ghi = [state.tile([128, BS], mmdt, name=f"ghi_{i}", tag=f"ghi_{i}")
                   for i in range(2)]
            Glo = [state.tile([128, BS], mmdt, name=f"glo_{i}", tag=f"glo_{i}")
                   for i in range(2)]

            nc.gpsimd.memset(x16[0][:], 0.0)
            if w_is_ones:
                nc.gpsimd.memset(xt[0][:], float(tau))
            else:
                nc.sync.dma_start(xt[0][:], TW_d[:])
            nc.vector.tensor_scalar_mul(gm[0][:], Bt[:], -1.0)
            nc.sync.dma_start(Ghi[0][:], GHI0_d[:])
            nc.sync.dma_start(Glo[0][:], GLO0_d[:])

            for t in range(N_ITERS):
                cur, nxt = t % 2, (t + 1) % 2

                # j = 0.5*gm - B  (early, DVE idle window during the MM phase)
                j = tmp.tile([128, BS], f32, tag="j", name="j")
                nc.vector.scalar_tensor_tensor(
                    j[:], gm[cur][:], 0.5, Bt[:], ALU.mult, ALU.subtract
                )

                # ---- dual: pV = S@x16^T + (Ghi + Glo)/2   (EYE = I/2)
                pV = psum.tile([128, BS], f32, tag="pV")
                for k in range(N_CHUNKS):
                    nc.tensor.matmul(
                        pV[:],
                        STs[:, 128 * k : 128 * (k + 1)],
                        x16[cur][:, BS * k : BS * (k + 1)],
                        start=(k == 0),
                        stop=False,
                    )
                nc.tensor.matmul(pV[:], EYE[:], Ghi[cur][:], start=False, stop=False)
                nc.tensor.matmul(pV[:], EYE[:], Glo[cur][:], start=False, stop=True)

                # critical: y16 = relu(2 sigma pV) in fp16
                y16 = tmp.tile([128, BS], mmdt, tag="y16", name="y16")
                nc.vector.tensor_scalar(
                    y16[:], pV[:], two_sigma, 0.0, ALU.mult, ALU.max
                )
                # gm' = |pV| + j, via |pV| = relu(2s pV)/s - pV
                y32f = tmp.tile([128, BS], f32, tag="y32f", name="y32f")
                nc.scalar.activation(
                    y32f[:], pV[:], ACT.Relu, bias=zb[:], scale=two_sigma
                )
                u = tmp.tile([128, BS], f32, tag="u", name="u")
                nc.vector.scalar_tensor_tensor(
                    u[:], pV[:], -1.0, j[:], ALU.mult, ALU.add
                )
                nc.vector.scalar_tensor_tensor(
                    gm[nxt][:], y32f[:], float(1.0 / sigma), u[:], ALU.mult, ALU.add
                )
                nc.scalar.activation(Ghi[nxt][:], gm[nxt][:], ACT.Copy)
                nc.gpsimd.tensor_sub(Glo[nxt][:], gm[nxt][:], Ghi[nxt][:])

                # ---- primal: pX[:, 32m:] = S^T y16
                pX = psum.tile([128, FD], f32, tag="pX")
                for m in range(N_CHUNKS):
                    nc.tensor.matmul(
                        pX[:, BS * m : BS * (m + 1)],
                        Ss[:, 128 * m : 128 * (m + 1)],
                        y16[:],
                        start=True,
                        stop=True,
                    )
                # e = xt - tau*pX ; x16' = relu(e) fp16 ; xt' = relu(e) + tau*W
                e = tmp.tile([128, FD], f32, tag="e", name="e")
                for h in range(2):
                    sl = slice(HF * h, HF * (h + 1))
                    nc.vector.scalar_tensor_tensor(
                        e[:, sl], pX[:, sl], float(-tau), xt[cur][:, sl],
                        ALU.mult, ALU.add,
                    )
                    nc.vector.tensor_scalar_max(x16[nxt][:, sl], e[:, sl], 0.0)
                    if w_is_ones:
                        # xt' = relu(e) + tau  (post-relu add: 2-op tensor_scalar)
                        nc.gpsimd.tensor_scalar(
                            xt[nxt][:, sl], e[:, sl], 0.0, float(tau),
                            ALU.max, ALU.add,
                        )
                    else:
                        xr = tmp.tile([128, HF], f32, tag=f"xr{h}", name=f"xr{h}")
                        nc.gpsimd.tensor_scalar_max(xr[:], e[:, sl], 0.0)
                        nc.gpsimd.tensor_add(xt[nxt][:, sl], xr[:], TW[:, sl])

            nc.sync.dma_start(XO_d[:], xt[N_ITERS % 2][:])

    return nc


MM_DTYPE = os.environ.get("GM_MM_DTYPE", "float32")


def kernel_run(X, S, W, batch_size, trace=False, tmpdir=None):
    from concourse.bass_utils import run_bass_kernel_spmd

    X = np.asarray(X, np.float32)
    S = np.asarray(S, np.float32)
    W = np.asarray(W, np.float32)
    bs = int(batch_size)
    assert bs == BATCH and X.shape == (BATCH, N_HOS, N_TYPES)
    assert S.shape == (M_CONS, N_STRUCTS)

    L = _spec_norm_f32(S)
    sigma = np.float32(0.9) / L
    tau = np.float32(0.9) / L

    B = X.reshape(BATCH, M_CONS)
    w_is_ones = bool(np.all(W == 1.0))

    np_mmdt = {"float32": np.float32, "float16": np.float16}[MM_DTYPE]
    # STs[p, 128k+j] = S[j, 128k+p]  (exact 0/1 in fp16)
    STs = (
        S.T.reshape(N_CHUNKS, 128, 128)
        .transpose(1, 0, 2)
        .reshape(128, N_STRUCTS)
        .astype(np_mmdt)
    )
    Ss = S.astype(np_mmdt)

    in_maps = []
    EYE = (0.5 * np.eye(128)).astype(np_mmdt)
    for c in range(N_CORES):
        Bt = np.ascontiguousarray(B[BS * c : BS * (c + 1), :].T.astype(np.float32))
        gm0 = -Bt
        GHI0 = gm0.astype(np_mmdt)
        GLO0 = (gm0 - GHI0.astype(np.float32)).astype(np_mmdt)
        m = {
            "STs": STs,
            "Ss": Ss,
            "EYE": EYE,
            "BH": Bt,
            "GHI0": np.ascontiguousarray(GHI0),
            "GLO0": np.ascontiguousarray(GLO0),
        }
        if not w_is_ones:
            TW_c = np.broadcast_to(
                (tau * W).reshape(N_CHUNKS, 128, 1), (N_CHUNKS, 128, BS)
            )
            m["TW"] = np.ascontiguousarray(
                TW_c.transpose(1, 0, 2).reshape(128, N_CHUNKS * BS).astype(np.float32)
            )
        in_maps.append(m)

    nc = _get_compiled(float(tau), float(sigma), MM_DTYPE, w_is_ones)
    res = run_bass_kernel_spmd(
        nc, in_maps, list(range(N_CORES)), trace=trace, tmpdir=tmpdir
    )

    out = np.empty((BATCH, N_STRUCTS), np.float32)
    for c in range(N_CORES):
        O = res.results[c]["XOUT"]  # [128, N_CHUNKS*BS]
        out[BS * c : BS * (c + 1), :] = (
            O.reshape(128, N_CHUNKS, BS).transpose(2, 1, 0).reshape(BS, N_STRUCTS)
        )
    out -= (tau * W)[None, :]
    return out, res


def kernel(**inputs):
    out, _ = kernel_run(
        inputs["X"], inputs["S"], inputs["W"], inputs["batch_size"], trace=False
    )
    return out


# revision 15
# speedup vs baseline: 2.4495x; 2.4495x over previous
"""Bass/Trainium2 kernel for nn_GreedyMatcher: batched PDHG LP solver.

Reference computation (per batch sample b):
    B = X.reshape(bs, 128); Wb = broadcast(W)
    x0 = y0 = 0, xbar0 = 0
    repeat 100x:
        y   = relu(y + sigma*(xbar @ S.T - B))
        x'  = relu(x + tau*(W - y @ S))
        xbar = 2x' - x ; x = x'
    return x  [bs, 2048]

Strategy: pure data parallel over batch (256 -> 32 per core, 8 cores).
Per-core state is kept struct-major in SBUF: Xsb[p, 32*m + b] = x[b, 128*m + p]
so both matmuls per iteration run with K=128 chunks on the tensor engine.
The extrapolation xbar is never materialized: with V_t = S @ x_t^T,
S @ xbar_t^T = 2 V_t - V_{t-1}, and the dual update folds into a single
carried tensor g_t = y_t - sigma*V_{t-1} - sigma*B^T:
    v      = (2 sigma V_t) + g_t          # psum + g
    y_.    = relu(v)
    g_{t+1}= y_. - sigma*V_t - sigma*B^T
    x_{t+1}= relu(x_t + tau - tau*(S^T y_.))   (W == ones fast path)
"""

import sys
import os

sys.path.insert(0, "/opt/trn_rl_repo")

import numpy as np

N_CORES = 8
BATCH = 256
BS = BATCH // N_CORES  # 32 per core
N_HOS = 8
N_TYPES = 16
M_CONS = N_HOS * N_TYPES  # 128 constraints
N_STRUCTS = 2048
N_CHUNKS = N_STRUCTS // 128  # 16
N_ITERS = 100
N_STREAMS = 1

_CACHE = {}


def _spec_norm_f32(S: np.ndarray) -> np.float32:
    """Mimic reference._spec_norm in float32 numpy."""
    S = S.astype(np.float32)
    v = np.ones((S.shape[1],), np.float32)
    v = v / np.float32(np.linalg.norm(v))
    for _ in range(30):
        u = S @ v
        u = u / (np.float32(np.linalg.norm(u)) + np.float32(1e-12))
        v = S.T @ u
        v = v / (np.float32(np.linalg.norm(v)) + np.float32(1e-12))
    return np.float32(np.linalg.norm(S @ v))


def _get_compiled(tau: float, sigma: float, mm_dtype_name: str, w_is_ones: bool):
    key = (round(float(tau), 12), round(float(sigma), 12), mm_dtype_name, w_is_ones)
    if key in _CACHE:
        return _CACHE[key]
    nc = _build_real(mm_dtype_name, w_is_ones, float(tau), float(sigma))
    nc.compile()
    _CACHE[key] = nc
    return nc


def _build_real(mm_dtype_name: str, w_is_ones: bool, tau: float, sigma: float):
    import concourse.bacc as bacc
    import concourse.tile as tile
    import concourse.mybir as mybir
    from contextlib import ExitStack

    f32 = mybir.dt.float32
    mmdt = getattr(mybir.dt, mm_dtype_name)
    ALU = mybir.AluOpType
    ACT = mybir.ActivationFunctionType

    nc = bacc.Bacc(None, target_bir_lowering=False)

    STs_d = nc.dram_tensor("STs", [128, N_STRUCTS], mmdt, kind="ExternalInput")
    Ss_d = nc.dram_tensor("Ss", [128, N_STRUCTS], mmdt, kind="ExternalInput")
    EYE_d = nc.dram_tensor("EYE", [128, 128], mmdt, kind="ExternalInput")
    BH_d = nc.dram_tensor("BH", [128, BS], f32, kind="ExternalInput")
    GHI0_d = nc.dram_tensor("GHI0", [128, BS], mmdt, kind="ExternalInput")
    GLO0_d = nc.dram_tensor("GLO0", [128, BS], mmdt, kind="ExternalInput")
    if not w_is_ones:
        TW_d = nc.dram_tensor("TW", [128, N_CHUNKS * BS], f32, kind="ExternalInput")
    XO_d = nc.dram_tensor("XOUT", [128, N_CHUNKS * BS], f32, kind="ExternalOutput")

    FD = N_CHUNKS * BS  # 512
    HF = FD // 2

    two_sigma = float(2.0 * sigma)

    with tile.TileContext(nc) as tc:
        with ExitStack() as ctx:
            const = ctx.enter_context(tc.tile_pool(name="const", bufs=1))
            state = ctx.enter_context(tc.tile_pool(name="state", bufs=1))
            tmp = ctx.enter_context(tc.tile_pool(name="tmp", bufs=3))
            psum = ctx.enter_context(tc.tile_pool(name="psum", bufs=2, space="PSUM"))

            STs = const.tile([128, N_STRUCTS], mmdt, tag="STs")
            Ss = const.tile([128, N_STRUCTS], mmdt, tag="Ss")
            EYE = const.tile([128, 128], mmdt, tag="EYE")
            Bt = const.tile([128, BS], f32, tag="Bt")
            nc.sync.dma_start(STs[:], STs_d[:])
            nc.sync.dma_start(Ss[:], Ss_d[:])
            nc.sync.dma_start(EYE[:], EYE_d[:])
            nc.sync.dma_start(Bt[:], BH_d[:])
            if not w_is_ones:
                TW = const.tile([128, FD], f32, tag="TW")
                nc.sync.dma_start(TW[:], TW_d[:])

            taub = const.tile([128, 1], f32, tag="taub")
            zb = const.tile([128, 1], f32, tag="zb")
            nc.gpsimd.memset(taub[:], float(tau))
            nc.gpsimd.memset(zb[:], 0.0)

            # state: xt = x + tau*W (fp32), x16 = fp16(x), gm = g/sigma (fp32)
            # plus fp16 hi/lo pair of gm for the PSUM identity fold.
            xt = [state.tile([128, FD], f32, name=f"xt_{i}", tag=f"xt_{i}")
                  for i in range(2)]
            x16 = [state.tile([128, FD], mmdt, name=f"x16_{i}", tag=f"x16_{i}")
                   for i in range(2)]
            gm = [state.tile([128, BS], f32, name=f"gm_{i}", tag=f"gm_{i}")
                  for i in range(2)]
            Ghi = [state.tile([128, BS], mmdt, name=f"ghi_{i}", tag=f"ghi_{i}")
                   for i in range(2)]
            Glo = [state.tile([128, BS], mmdt, name=f"glo_{i}", tag=f"glo_{i}")
                   for i in range(2)]

            nc.gpsimd.memset(x16[0][:], 0.0)
            nc.gpsimd.memset(xt[0][:], 0.0)
            nc.vector.tensor_scalar_mul(gm[0][:], Bt[:], -1.0)
            nc.sync.dma_start(Ghi[0][:], GHI0_d[:])
            nc.sync.dma_start(Glo[0][:], GLO0_d[:])

            for t in range(N_ITERS):
                cur, nxt = t % 2, (t + 1) % 2

                # j = 0.5*gm - B  (early, DVE idle window during the MM phase)
                j = tmp.tile([128, BS], f32, tag="j", name="j")
                nc.vector.scalar_tensor_tensor(
                    j[:], gm[cur][:], 0.5, Bt[:], ALU.mult, ALU.subtract
                )

                # ---- dual: pV = S@x16^T + (Ghi + Glo)/2   (EYE = I/2)
                pV = psum.tile([128, BS], f32, tag="pV")
                for k in range(N_CHUNKS):
                    nc.tensor.matmul(
                        pV[:],
                        STs[:, 128 * k : 128 * (k + 1)],
                        x16[cur][:, BS * k : BS * (k + 1)],
                        start=(k == 0),
                        stop=False,
                    )
                nc.tensor.matmul(pV[:], EYE[:], Ghi[cur][:], start=False, stop=False)
                nc.tensor.matmul(pV[:], EYE[:], Glo[cur][:], start=False, stop=True)

                # critical: y16 = relu(2 sigma pV) in fp16
                y16 = tmp.tile([128, BS], mmdt, tag="y16", name="y16")
                nc.vector.tensor_scalar(
                    y16[:], pV[:], two_sigma, 0.0, ALU.mult, ALU.max
                )
                # gm' = |pV| + j, via |pV| = relu(2s pV)/s - pV
                y32f = tmp.tile([128, BS], f32, tag="y32f", name="y32f")
                nc.scalar.activation(
                    y32f[:], pV[:], ACT.Relu, bias=zb[:], scale=two_sigma
                )
                u = tmp.tile([128, BS], f32, tag="u", name="u")
                nc.vector.scalar_tensor_tensor(
                    u[:], pV[:], -1.0, j[:], ALU.mult, ALU.add
                )
                nc.vector.scalar_tensor_tensor(
                    gm[nxt][:], y32f[:], float(1.0 / sigma), u[:], ALU.mult, ALU.add
                )
                nc.scalar.activation(Ghi[nxt][:], gm[nxt][:], ACT.Copy)
                nc.gpsimd.tensor_sub(Glo[nxt][:], gm[nxt][:], Ghi[nxt][:])

                # ---- primal: pX[:, 32m:] = S^T y16
                pX = psum.tile([128, FD], f32, tag="pX")
                for m in range(N_CHUNKS):
                    nc.tensor.matmul(
                        pX[:, BS * m : BS * (m + 1)],
                        Ss[:, 128 * m : 128 * (m + 1)],
                        y16[:],
                        start=True,
                        stop=True,
                    )
                # e = x - tau*pX ; x' = relu(e + tau*W)
                e = tmp.tile([128, FD], f32, tag="e", name="e")
                for h in range(2):
                    sl = slice(HF * h, HF * (h + 1))
                    nc.vector.scalar_tensor_tensor(
                        e[:, sl], pX[:, sl], float(-tau), xt[cur][:, sl],
                        ALU.mult, ALU.add,
                    )
                    if w_is_ones:
                        nc.vector.tensor_scalar(
                            x16[nxt][:, sl], e[:, sl], float(tau), 0.0,
                            ALU.add, ALU.max,
                        )
                        nc.scalar.activation(
                            xt[nxt][:, sl], e[:, sl], ACT.Relu, bias=taub[:]
                        )
                    else:
                        e2 = tmp.tile([128, HF], f32, tag=f"e2{h}", name=f"e2{h}")
                        nc.vector.tensor_add(e2[:], e[:, sl], TW[:, sl])
                        nc.vector.tensor_scalar_max(x16[nxt][:, sl], e2[:], 0.0)
                        nc.scalar.activation(
                            xt[nxt][:, sl], e2[:], ACT.Relu, bias=zb[:]
                        )

            nc.sync.dma_start(XO_d[:], xt[N_ITERS % 2][:])

    return nc


MM_DTYPE = os.environ.get("GM_MM_DTYPE", "float32")


def kernel_run(X, S, W, batch_size, trace=False, tmpdir=None):
    from concourse.bass_utils import run_bass_kernel_spmd

    X = np.asarray(X, np.float32)
    S = np.asarray(S, np.float32)
    W = np.asarray(W, np.float32)
    bs = int(batch_size)
    assert bs == BATCH and X.shape == (BATCH, N_HOS, N_TYPES)
    assert S.shape == (M_CONS, N_STRUCTS)

    L = _spec_norm_f32(S)
    sigma = np.float32(0.9) / L
    tau = np.float32(0.9) / L

    B = X.reshape(BATCH, M_CONS)
    w_is_ones = bool(np.all(W == 1.0))

    np_mmdt = {"float32": np.float32, "float16": np.float16}[MM_DTYPE]
    # STs[p, 128k+j] = S[j, 128k+p]  (exact 0/1 in fp16)
    STs = (
        S.T.reshape(N_CHUNKS, 128, 128)
        .transpose(1, 0, 2)
        .reshape(128, N_STRUCTS)
        .astype(np_mmdt)
    )
    Ss = S.astype(np_mmdt)

    in_maps = []
    EYE = (0.5 * np.eye(128)).astype(np_mmdt)
    for c in range(N_CORES):
        Bt = np.ascontiguousarray(B[BS * c : BS * (c + 1), :].T.astype(np.float32))
        gm0 = -Bt
        GHI0 = gm0.astype(np_mmdt)
        GLO0 = (gm0 - GHI0.astype(np.float32)).astype(np_mmdt)
        m = {
            "STs": STs,
            "Ss": Ss,
            "EYE": EYE,
            "BH": Bt,
            "GHI0": np.ascontiguousarray(GHI0),
            "GLO0": np.ascontiguousarray(GLO0),
        }
        if not w_is_ones:
            TW_c = np.broadcast_to(
                (tau * W).reshape(N_CHUNKS, 128, 1), (N_CHUNKS, 128, BS)
            )
            m["TW"] = np.ascontiguousarray(
                TW_c.transpose(1, 0, 2).reshape(128, N_CHUNKS * BS).astype(np.float32)
            )
        in_maps.append(m)

    nc = _get_compiled(float(tau), float(sigma), MM_DTYPE, w_is_ones)
    res = run_bass_kernel_spmd(
        nc, in_maps, list(range(N_CORES)), trace=trace, tmpdir=tmpdir
    )

    out = np.empty((BATCH, N_STRUCTS), np.float32)
    for c in range(N_CORES):
        O = res.results[c]["XOUT"]  # [128, N_CHUNKS*BS]
        out[BS * c : BS * (c + 1), :] = (
            O.reshape(128, N_CHUNKS, BS).transpose(2, 1, 0).reshape(BS, N_STRUCTS)
        )
    return out, res


def kernel(**inputs):
    out, _ = kernel_run(
        inputs["X"], inputs["S"], inputs["W"], inputs["batch_size"], trace=False
    )
    return out


# revision 17
# speedup vs baseline: 2.4931x; 1.0178x over previous
"""Bass/Trainium2 kernel for nn_GreedyMatcher: batched PDHG LP solver.

Reference computation (per batch sample b):
    B = X.reshape(bs, 128); Wb = broadcast(W)
    x0 = y0 = 0, xbar0 = 0
    repeat 100x:
        y   = relu(y + sigma*(xbar @ S.T - B))
        x'  = relu(x + tau*(W - y @ S))
        xbar = 2x' - x ; x = x'
    return x  [bs, 2048]

Strategy: pure data parallel over batch (256 -> 32 per core, 8 cores).
Per-core state is kept struct-major in SBUF: Xsb[p, 32*m + b] = x[b, 128*m + p]
so both matmuls per iteration run with K=128 chunks on the tensor engine.
The extrapolation xbar is never materialized: with V_t = S @ x_t^T,
S @ xbar_t^T = 2 V_t - V_{t-1}, and the dual update folds into a single
carried tensor g_t = y_t - sigma*V_{t-1} - sigma*B^T:
    v      = (2 sigma V_t) + g_t          # psum + g
    y_.    = relu(v)
    g_{t+1}= y_. - sigma*V_t - sigma*B^T
    x_{t+1}= relu(x_t + tau - tau*(S^T y_.))   (W == ones fast path)
"""

import sys
import os

sys.path.insert(0, "/opt/trn_rl_repo")

import numpy as np

N_CORES = 8
BATCH = 256
BS = BATCH // N_CORES  # 32 per core
N_HOS = 8
N_TYPES = 16
M_CONS = N_HOS * N_TYPES  # 128 constraints
N_STRUCTS = 2048
N_CHUNKS = N_STRUCTS // 128  # 16
N_ITERS = 100
N_STREAMS = 1

_CACHE = {}


def _spec_norm_f32(S: np.ndarray) -> np.float32:
    """Mimic reference._spec_norm in float32 numpy."""
    S = S.astype(np.float32)
    v = np.ones((S.shape[1],), np.float32)
    v = v / np.float32(np.linalg.norm(v))
    for _ in range(30):
        u = S @ v
        u = u / (np.float32(np.linalg.norm(u)) + np.float32(1e-12))
        v = S.T @ u
        v = v / (np.float32(np.linalg.norm(v)) + np.float32(1e-12))
    return np.float32(np.linalg.norm(S @ v))


def _get_compiled(tau: float, sigma: float, mm_dtype_name: str, w_is_ones: bool):
    key = (round(float(tau), 12), round(float(sigma), 12), mm_dtype_name, w_is_ones)
    if key in _CACHE:
        return _CACHE[key]
    nc = _build_real(mm_dtype_name, w_is_ones, float(tau), float(sigma))
    nc.compile()
    _CACHE[key] = nc
    return nc


def _build_real(mm_dtype_name: str, w_is_ones: bool, tau: float, sigma: float):
    import concourse.bacc as bacc
    import concourse.tile as tile
    import concourse.mybir as mybir
    from contextlib import ExitStack

    f32 = mybir.dt.float32
    mmdt = getattr(mybir.dt, mm_dtype_name)
    ALU = mybir.AluOpType
    ACT = mybir.ActivationFunctionType

    nc = bacc.Bacc(None, target_bir_lowering=False)

    STs_d = nc.dram_tensor("STs", [128, N_STRUCTS], mmdt, kind="ExternalInput")
    Ss_d = nc.dram_tensor("Ss", [128, N_STRUCTS], mmdt, kind="ExternalInput")
    EYE_d = nc.dram_tensor("EYE", [128, 128], mmdt, kind="ExternalInput")
    BH_d = nc.dram_tensor("BH", [128, BS], f32, kind="ExternalInput")
    GHI0_d = nc.dram_tensor("GHI0", [128, BS], mmdt, kind="ExternalInput")
    GLO0_d = nc.dram_tensor("GLO0", [128, BS], mmdt, kind="ExternalInput")
    if not w_is_ones:
        TW_d = nc.dram_tensor("TW", [128, N_CHUNKS * BS], f32, kind="ExternalInput")
    XO_d = nc.dram_tensor("XOUT", [128, N_CHUNKS * BS], f32, kind="ExternalOutput")

    FD = N_CHUNKS * BS  # 512
    HF = FD // 2

    two_sigma = float(2.0 * sigma)

    with tile.TileContext(nc) as tc:
        with ExitStack() as ctx:
            const = ctx.enter_context(tc.tile_pool(name="const", bufs=1))
            state = ctx.enter_context(tc.tile_pool(name="state", bufs=1))
            tmp = ctx.enter_context(tc.tile_pool(name="tmp", bufs=3))
            psum = ctx.enter_context(tc.tile_pool(name="psum", bufs=2, space="PSUM"))

            STs = const.tile([128, N_STRUCTS], mmdt, tag="STs")
            Ss = const.tile([128, N_STRUCTS], mmdt, tag="Ss")
            EYE = const.tile([128, 128], mmdt, tag="EYE")
            Bt = const.tile([128, BS], f32, tag="Bt")
            nc.sync.dma_start(STs[:], STs_d[:])
            nc.sync.dma_start(Ss[:], Ss_d[:])
            nc.sync.dma_start(EYE[:], EYE_d[:])
            nc.sync.dma_start(Bt[:], BH_d[:])
            if not w_is_ones:
                TW = const.tile([128, FD], f32, tag="TW")
                nc.sync.dma_start(TW[:], TW_d[:])

            taub = const.tile([128, 1], f32, tag="taub")
            zb = const.tile([128, 1], f32, tag="zb")
            nc.gpsimd.memset(taub[:], float(tau))
            nc.gpsimd.memset(zb[:], 0.0)

            # state: xt = x + tau*W (fp32), x16 = fp16(x), gm = g/sigma (fp32)
            # plus fp16 hi/lo pair of gm for the PSUM identity fold.
            xt = [state.tile([128, FD], f32, name=f"xt_{i}", tag=f"xt_{i}")
                  for i in range(2)]
            x16 = [state.tile([128, FD], mmdt, name=f"x16_{i}", tag=f"x16_{i}")
                   for i in range(2)]
            gm = [state.tile([128, BS], f32, name=f"gm_{i}", tag=f"gm_{i}")
                  for i in range(2)]
            Ghi = [state.tile([128, BS], mmdt, name=f"ghi_{i}", tag=f"ghi_{i}")
                   for i in range(2)]
            Glo = [state.tile([128, BS], mmdt, name=f"glo_{i}", tag=f"glo_{i}")
                   for i in range(2)]

            nc.gpsimd.memset(x16[0][:], 0.0)
            nc.gpsimd.memset(xt[0][:], 0.0)
            nc.vector.tensor_scalar_mul(gm[0][:], Bt[:], -1.0)
            nc.sync.dma_start(Ghi[0][:], GHI0_d[:])
            nc.sync.dma_start(Glo[0][:], GLO0_d[:])

            for t in range(N_ITERS):
                cur, nxt = t % 2, (t + 1) % 2

                # j = 0.5*gm - B  (early, DVE idle window during the MM phase)
                j = tmp.tile([128, BS], f32, tag="j", name="j")
                nc.vector.scalar_tensor_tensor(
                    j[:], gm[cur][:], 0.5, Bt[:], ALU.mult, ALU.subtract
                )

                # ---- dual: pV = S@x16^T + (Ghi + Glo)/2   (EYE = I/2)
                pV = psum.tile([128, BS], f32, tag="pV")
                for k in range(N_CHUNKS):
                    nc.tensor.matmul(
                        pV[:],
                        STs[:, 128 * k : 128 * (k + 1)],
                        x16[cur][:, BS * k : BS * (k + 1)],
                        start=(k == 0),
                        stop=False,
                    )
                nc.tensor.matmul(pV[:], EYE[:], Ghi[cur][:], start=False, stop=False)
                nc.tensor.matmul(pV[:], EYE[:], Glo[cur][:], start=False, stop=True)

                # critical: y16 = relu(2 sigma pV) in fp16
                y16 = tmp.tile([128, BS], mmdt, tag="y16", name="y16")
                nc.vector.tensor_scalar(
                    y16[:], pV[:], two_sigma, 0.0, ALU.mult, ALU.max
                )
                # ---- primal: pX[:, 32m:] = S^T y16
                pX = psum.tile([128, FD], f32, tag="pX")
                for m in range(N_CHUNKS):
                    nc.tensor.matmul(
                        pX[:, BS * m : BS * (m + 1)],
                        Ss[:, 128 * m : 128 * (m + 1)],
                        y16[:],
                        start=True,
                        stop=True,
                    )
                # e = x - tau*pX ; x' = relu(e + tau*W)
                e = tmp.tile([128, FD], f32, tag="e", name="e")
                for h in range(2):
                    sl = slice(HF * h, HF * (h + 1))
                    nc.vector.scalar_tensor_tensor(
                        e[:, sl], pX[:, sl], float(-tau), xt[cur][:, sl],
                        ALU.mult, ALU.add,
                    )
                    if w_is_ones:
                        nc.vector.tensor_scalar(
                            x16[nxt][:, sl], e[:, sl], float(tau), 0.0,
                            ALU.add, ALU.max,
                        )
                        nc.scalar.activation(
                            xt[nxt][:, sl], e[:, sl], ACT.Relu, bias=taub[:]
                        )
                    else:
                        e2 = tmp.tile([128, HF], f32, tag=f"e2{h}", name=f"e2{h}")
                        nc.vector.tensor_add(e2[:], e[:, sl], TW[:, sl])
                        nc.vector.tensor_scalar_max(x16[nxt][:, sl], e2[:], 0.0)
                        nc.scalar.activation(
                            xt[nxt][:, sl], e2[:], ACT.Relu, bias=zb[:]
                        )

                # gm' = |pV| + j via |pV| = relu(pV) + relu(-pV); all off DVE,
                # emitted late so the DVE queue holds only the critical chain.
                rp = tmp.tile([128, BS], f32, tag="rp", name="rp")
                nc.scalar.activation(rp[:], pV[:], ACT.Relu, bias=zb[:], scale=1.0)
                rn = tmp.tile([128, BS], f32, tag="rn", name="rn")
                nc.scalar.activation(rn[:], pV[:], ACT.Relu, bias=zb[:], scale=-1.0)
                m1 = tmp.tile([128, BS], f32, tag="m1", name="m1")
                nc.gpsimd.tensor_add(m1[:], rp[:], rn[:])
                nc.gpsimd.tensor_add(gm[nxt][:], m1[:], j[:])
                nc.gpsimd.tensor_copy(Ghi[nxt][:], gm[nxt][:])
                nc.gpsimd.tensor_sub(Glo[nxt][:], gm[nxt][:], Ghi[nxt][:])

            nc.sync.dma_start(XO_d[:], xt[N_ITERS % 2][:])

    return nc


MM_DTYPE = os.environ.get("GM_MM_DTYPE", "float32")


def kernel_run(X, S, W, batch_size, trace=False, tmpdir=None):
    from concourse.bass_utils import run_bass_kernel_spmd

    X = np.asarray(X, np.float32)
    S = np.asarray(S, np.float32)
    W = np.asarray(W, np.float32)
    bs = int(batch_size)
    assert bs == BATCH and X.shape == (BATCH, N_HOS, N_TYPES)
    assert S.shape == (M_CONS, N_STRUCTS)

    L = _spec_norm_f32(S)
    sigma = np.float32(0.9) / L
    tau = np.float32(0.9) / L

    B = X.reshape(BATCH, M_CONS)
    w_is_ones = bool(np.all(W == 1.0))

    np_mmdt = {"float32": np.float32, "float16": np.float16}[MM_DTYPE]
    # STs[p, 128k+j] = S[j, 128k+p]  (exact 0/1 in fp16)
    STs = (
        S.T.reshape(N_CHUNKS, 128, 128)
        .transpose(1, 0, 2)
        .reshape(128, N_STRUCTS)
        .astype(np_mmdt)
    )
    Ss = S.astype(np_mmdt)

    in_maps = []
    EYE = (0.5 * np.eye(128)).astype(np_mmdt)
    for c in range(N_CORES):
        Bt = np.ascontiguousarray(B[BS * c : BS * (c + 1), :].T.astype(np.float32))
        gm0 = -Bt
        GHI0 = gm0.astype(np_mmdt)
        GLO0 = (gm0 - GHI0.astype(np.float32)).astype(np_mmdt)
        m = {
            "STs": STs,
            "Ss": Ss,
            "EYE": EYE,
            "BH": Bt,
            "GHI0": np.ascontiguousarray(GHI0),
            "GLO0": np.ascontiguousarray(GLO0),
        }
        if not w_is_ones:
            TW_c = np.broadcast_to(
                (tau * W).reshape(N_CHUNKS, 128, 1), (N_CHUNKS, 128, BS)
            )
            m["TW"] = np.ascontiguousarray(
                TW_c.transpose(1, 0, 2).reshape(128, N_CHUNKS * BS).astype(np.float32)
            )
        in_maps.append(m)

    nc = _get_compiled(float(tau), float(sigma), MM_DTYPE, w_is_ones)
    res = run_bass_kernel_spmd(
        nc, in_maps, list(range(N_CORES)), trace=trace, tmpdir=tmpdir
    )

    out = np.empty((BATCH, N_STRUCTS), np.float32)
    for c in range(N_CORES):
        O = res.results[c]["XOUT"]  # [128, N_CHUNKS*BS]
        out[BS * c : BS * (c + 1), :] = (
            O.reshape(128, N_CHUNKS, BS).transpose(2, 1, 0).reshape(BS, N_STRUCTS)
        )
    return out, res


def kernel(**inputs):
    out, _ = kernel_run(
        inputs["X"], inputs["S"], inputs["W"], inputs["batch_size"], trace=False
    )
    return out


# revision 18
# speedup vs baseline: 2.9005x; 1.1634x over previous
"""Bass/Trainium2 kernel for nn_GreedyMatcher: batched PDHG LP solver.

Reference computation (per batch sample b):
    B = X.reshape(bs, 128); Wb = broadcast(W)
    x0 = y0 = 0, xbar0 = 0
    repeat 100x:
        y   = relu(y + sigma*(xbar @ S.T - B))
        x'  = relu(x + tau*(W - y @ S))
        xbar = 2x' - x ; x = x'
    return x  [bs, 2048]

Strategy: pure data parallel over batch (256 -> 32 per core, 8 cores).
Per-core state is kept struct-major in SBUF: Xsb[p, 32*m + b] = x[b, 128*m + p]
so both matmuls per iteration run with K=128 chunks on the tensor engine.
The extrapolation xbar is never materialized: with V_t = S @ x_t^T,
S @ xbar_t^T = 2 V_t - V_{t-1}, and the dual update folds into a single
carried tensor g_t = y_t - sigma*V_{t-1} - sigma*B^T:
    v      = (2 sigma V_t) + g_t          # psum + g
    y_.    = relu(v)
    g_{t+1}= y_. - sigma*V_t - sigma*B^T
    x_{t+1}= relu(x_t + tau - tau*(S^T y_.))   (W == ones fast path)
"""

import sys
import os

sys.path.insert(0, "/opt/trn_rl_repo")

import numpy as np

N_CORES = 8
BATCH = 256
BS = BATCH // N_CORES  # 32 per core
N_HOS = 8
N_TYPES = 16
M_CONS = N_HOS * N_TYPES  # 128 constraints
N_STRUCTS = 2048
N_CHUNKS = N_STRUCTS // 128  # 16
N_ITERS = 100
N_STREAMS = 1

_CACHE = {}


def _spec_norm_f32(S: np.ndarray) -> np.float32:
    """Mimic reference._spec_norm in float32 numpy."""
    S = S.astype(np.float32)
    v = np.ones((S.shape[1],), np.float32)
    v = v / np.float32(np.linalg.norm(v))
    for _ in range(30):
        u = S @ v
        u = u / (np.float32(np.linalg.norm(u)) + np.float32(1e-12))
        v = S.T @ u
        v = v / (np.float32(np.linalg.norm(v)) + np.float32(1e-12))
    return np.float32(np.linalg.norm(S @ v))


def _get_compiled(tau: float, sigma: float, mm_dtype_name: str, w_is_ones: bool):
    key = (round(float(tau), 12), round(float(sigma), 12), mm_dtype_name, w_is_ones)
    if key in _CACHE:
        return _CACHE[key]
    nc = _build_real(mm_dtype_name, w_is_ones, float(tau), float(sigma))
    nc.compile()
    _CACHE[key] = nc
    return nc


def _build_real(mm_dtype_name: str, w_is_ones: bool, tau: float, sigma: float):
    import concourse.bacc as bacc
    import concourse.tile as tile
    import concourse.mybir as mybir
    from contextlib import ExitStack

    f32 = mybir.dt.float32
    mmdt = getattr(mybir.dt, mm_dtype_name)
    ALU = mybir.AluOpType
    ACT = mybir.ActivationFunctionType

    nc = bacc.Bacc(None, target_bir_lowering=False)

    STs_d = nc.dram_tensor("STs", [128, N_STRUCTS], mmdt, kind="ExternalInput")
    Ss_d = nc.dram_tensor("Ss", [128, N_STRUCTS], mmdt, kind="ExternalInput")
    EYE_d = nc.dram_tensor("EYE", [128, 128], mmdt, kind="ExternalInput")
    BH_d = nc.dram_tensor("BH", [128, BS], f32, kind="ExternalInput")
    GHI0_d = nc.dram_tensor("GHI0", [128, BS], mmdt, kind="ExternalInput")
    GLO0_d = nc.dram_tensor("GLO0", [128, BS], mmdt, kind="ExternalInput")
    if not w_is_ones:
        TW_d = nc.dram_tensor("TW", [128, N_CHUNKS * BS], f32, kind="ExternalInput")
    XO_d = nc.dram_tensor("XOUT", [128, N_CHUNKS * BS], f32, kind="ExternalOutput")

    FD = N_CHUNKS * BS  # 512
    HF = FD // 2

    two_sigma = float(2.0 * sigma)

    with tile.TileContext(nc) as tc:
        with ExitStack() as ctx:
            const = ctx.enter_context(tc.tile_pool(name="const", bufs=1))
            state = ctx.enter_context(tc.tile_pool(name="state", bufs=1))
            tmp = ctx.enter_context(tc.tile_pool(name="tmp", bufs=3))
            psum = ctx.enter_context(tc.tile_pool(name="psum", bufs=2, space="PSUM"))

            STs = const.tile([128, N_STRUCTS], mmdt, tag="STs")
            Ss = const.tile([128, N_STRUCTS], mmdt, tag="Ss")
            EYE = const.tile([128, 128], mmdt, tag="EYE")
            Bt = const.tile([128, BS], f32, tag="Bt")
            nc.sync.dma_start(STs[:], STs_d[:])
            nc.sync.dma_start(Ss[:], Ss_d[:])
            nc.sync.dma_start(EYE[:], EYE_d[:])
            nc.sync.dma_start(Bt[:], BH_d[:])
            if not w_is_ones:
                TW = const.tile([128, FD], f32, tag="TW")
                nc.sync.dma_start(TW[:], TW_d[:])

            taub = const.tile([128, 1], f32, tag="taub")
            zb = const.tile([128, 1], f32, tag="zb")
            nc.gpsimd.memset(taub[:], float(tau))
            nc.gpsimd.memset(zb[:], 0.0)

            # state: xt = x + tau*W (fp32), x16 = fp16(x), gm = g/sigma (fp32)
            # plus fp16 hi/lo pair of gm for the PSUM identity fold.
            xt = [state.tile([128, FD], f32, name=f"xt_{i}", tag=f"xt_{i}")
                  for i in range(2)]
            x16 = [state.tile([128, FD], mmdt, name=f"x16_{i}", tag=f"x16_{i}")
                   for i in range(2)]
            gm = [state.tile([128, BS], f32, name=f"gm_{i}", tag=f"gm_{i}")
                  for i in range(2)]
            Ghi = [state.tile([128, BS], mmdt, name=f"ghi_{i}", tag=f"ghi_{i}")
                   for i in range(2)]
            Glo = [state.tile([128, BS], mmdt, name=f"glo_{i}", tag=f"glo_{i}")
                   for i in range(2)]

            nc.gpsimd.memset(x16[0][:], 0.0)
            nc.gpsimd.memset(xt[0][:], 0.0)
            nc.vector.tensor_scalar_mul(gm[0][:], Bt[:], -1.0)
            nc.sync.dma_start(Ghi[0][:], GHI0_d[:])
            nc.sync.dma_start(Glo[0][:], GLO0_d[:])

            for t in range(N_ITERS):
                cur, nxt = t % 2, (t + 1) % 2

                # ---- dual: pV = S@x16^T + (Ghi + Glo)/2   (EYE = I/2)
                pV = psum.tile([128, BS], f32, tag="pV")
                for k in range(N_CHUNKS):
                    nc.tensor.matmul(
                        pV[:],
                        STs[:, 128 * k : 128 * (k + 1)],
                        x16[cur][:, BS * k : BS * (k + 1)],
                        start=(k == 0),
                        stop=False,
                    )
                nc.tensor.matmul(pV[:], EYE[:], Ghi[cur][:], start=False, stop=False)
                nc.tensor.matmul(pV[:], EYE[:], Glo[cur][:], start=False, stop=True)

                # critical: y16 = relu(2 sigma pV) in fp16
                y16 = tmp.tile([128, BS], mmdt, tag="y16", name="y16")
                nc.vector.tensor_scalar(
                    y16[:], pV[:], two_sigma, 0.0, ALU.mult, ALU.max
                )
                # j = 0.5*gm - B (fills the DVE idle slot right after y16)
                j = tmp.tile([128, BS], f32, tag="j", name="j")
                nc.vector.scalar_tensor_tensor(
                    j[:], gm[cur][:], 0.5, Bt[:], ALU.mult, ALU.subtract
                )
                # ---- primal: pX[:, 32m:] = S^T y16
                pXh = [psum.tile([128, HF], f32, tag=f"pX{i}", name=f"pX{i}")
                       for i in range(2)]
                HC = N_CHUNKS // 2
                for m in range(N_CHUNKS):
                    nc.tensor.matmul(
                        pXh[m // HC][:, BS * (m % HC) : BS * (m % HC + 1)],
                        Ss[:, 128 * m : 128 * (m + 1)],
                        y16[:],
                        start=True,
                        stop=True,
                    )
                # e = x - tau*pX ; x' = relu(e + tau*W)
                e = tmp.tile([128, FD], f32, tag="e", name="e")
                for h in range(2):
                    sl = slice(HF * h, HF * (h + 1))
                    nc.vector.scalar_tensor_tensor(
                        e[:, sl], pXh[h][:], float(-tau), xt[cur][:, sl],
                        ALU.mult, ALU.add,
                    )
                    if w_is_ones:
                        nc.vector.tensor_scalar(
                            x16[nxt][:, sl], e[:, sl], float(tau), 0.0,
                            ALU.add, ALU.max,
                        )
                        nc.scalar.activation(
                            xt[nxt][:, sl], e[:, sl], ACT.Relu, bias=taub[:]
                        )
                    else:
                        e2 = tmp.tile([128, HF], f32, tag=f"e2{h}", name=f"e2{h}")
                        nc.vector.tensor_add(e2[:], e[:, sl], TW[:, sl])
                        nc.vector.tensor_scalar_max(x16[nxt][:, sl], e2[:], 0.0)
                        nc.scalar.activation(
                            xt[nxt][:, sl], e2[:], ACT.Relu, bias=zb[:]
                        )

                # gm' = |pV| + j via |pV| = relu(pV) + relu(-pV); all off DVE,
                # emitted late so the DVE queue holds only the critical chain.
                rp = tmp.tile([128, BS], f32, tag="rp", name="rp")
                nc.scalar.activation(rp[:], pV[:], ACT.Relu, bias=zb[:], scale=1.0)
                rn = tmp.tile([128, BS], f32, tag="rn", name="rn")
                nc.scalar.activation(rn[:], pV[:], ACT.Relu, bias=zb[:], scale=-1.0)
                m1 = tmp.tile([128, BS], f32, tag="m1", name="m1")
                nc.gpsimd.tensor_add(m1[:], rp[:], rn[:])
                nc.gpsimd.tensor_add(gm[nxt][:], m1[:], j[:])
                nc.gpsimd.tensor_copy(Ghi[nxt][:], gm[nxt][:])
                nc.gpsimd.tensor_sub(Glo[nxt][:], gm[nxt][:], Ghi[nxt][:])

            nc.sync.dma_start(XO_d[:], xt[N_ITERS % 2][:])

    return nc


MM_DTYPE = os.environ.get("GM_MM_DTYPE", "float32")


def kernel_run(X, S, W, batch_size, trace=False, tmpdir=None):
    from concourse.bass_utils import run_bass_kernel_spmd

    X = np.asarray(X, np.float32)
    S = np.asarray(S, np.float32)
    W = np.asarray(W, np.float32)
    bs = int(batch_size)
    assert bs == BATCH and X.shape == (BATCH, N_HOS, N_TYPES)
    assert S.shape == (M_CONS, N_STRUCTS)

    L = _spec_norm_f32(S)
    sigma = np.float32(0.9) / L
    tau = np.float32(0.9) / L

    B = X.reshape(BATCH, M_CONS)
    w_is_ones = bool(np.all(W == 1.0))

    np_mmdt = {"float32": np.float32, "float16": np.float16}[MM_DTYPE]
    # STs[p, 128k+j] = S[j, 128k+p]  (exact 0/1 in fp16)
    STs = (
        S.T.reshape(N_CHUNKS, 128, 128)
        .transpose(1, 0, 2)
        .reshape(128, N_STRUCTS)
        .astype(np_mmdt)
    )
    Ss = S.astype(np_mmdt)

    in_maps = []
    EYE = (0.5 * np.eye(128)).astype(np_mmdt)
    for c in range(N_CORES):
        Bt = np.ascontiguousarray(B[BS * c : BS * (c + 1), :].T.astype(np.float32))
        gm0 = -Bt
        GHI0 = gm0.astype(np_mmdt)
        GLO0 = (gm0 - GHI0.astype(np.float32)).astype(np_mmdt)
        m = {
            "STs": STs,
            "Ss": Ss,
            "EYE": EYE,
            "BH": Bt,
            "GHI0": np.ascontiguousarray(GHI0),
            "GLO0": np.ascontiguousarray(GLO0),
        }
        if not w_is_ones:
            TW_c = np.broadcast_to(
                (tau * W).reshape(N_CHUNKS, 128, 1), (N_CHUNKS, 128, BS)
            )
            m["TW"] = np.ascontiguousarray(
                TW_c.transpose(1, 0, 2).reshape(128, N_CHUNKS * BS).astype(np.float32)
            )
        in_maps.append(m)

    nc = _get_compiled(float(tau), float(sigma), MM_DTYPE, w_is_ones)
    res = run_bass_kernel_spmd(
        nc, in_maps, list(range(N_CORES)), trace=trace, tmpdir=tmpdir
    )

    out = np.empty((BATCH, N_STRUCTS), np.float32)
    for c in range(N_CORES):
        O = res.results[c]["XOUT"]  # [128, N_CHUNKS*BS]
        out[BS * c : BS * (c + 1), :] = (
            O.reshape(128, N_CHUNKS, BS).transpose(2, 1, 0).reshape(BS, N_STRUCTS)
        )
    return out, res


def kernel(**inputs):
    out, _ = kernel_run(
        inputs["X"], inputs["S"], inputs["W"], inputs["batch_size"], trace=False
    )
    return out
